# revision 29
# baseline (speedup 1.0000x reference)
"""Trainium2 Bass kernel for a 3-layer GAT forward pass (B=8, N=1024).

Sharding: data-parallel over batch B across 8 NeuronCores (one batch
element per core); parameters replicated. Per core everything runs in a
"transposed" layout ([feature/key-node j on partitions, query-node i on
the free dim]) so the softmax aggregation matmuls need no NxN transposes.

Per (j-chunk, head) tile [128, 1024]:
    sij  = si_repl + s_j[j]            (GPS tensor_scalar, per-partition AP)
    z    = a_e*adjT + sij              (DVE scalar_tensor_tensor)
    w    = max(z, 0.2*z)               (DVE stt: leaky_relu)
    ex   = exp(w)                      (ACT; |z|<=~10 so no max-subtract)
    s   += ones^T @ ex                 (PE)
    aggT+= h_chunk^T @ ex              (PE, contraction over j)
adj==0 masking: the input adj has only a couple of exact zeros (union
over batches), so those positions are baked into the program as tiny
patches: ex[p] *= (this core's adj[p] != 0), computed via 4-byte DMAs
plus a [1,1] DVE multiply (exact per-core softmax masking).

Layer-3 softmax normalization of the [N,N,H] attention output and the
final axis re-orderings run host-side in numpy (cheap glue; all O(N^2)
compute stays on device).
"""

import numpy as np
from contextlib import ExitStack

import concourse.bass as bass
import concourse.tile as tile
import concourse.mybir as mybir
from concourse.alu_op_type import AluOpType as alu
import bass_rust

# ---------------------------------------------------------------------------
# Workarounds for the neuronxcc build in this container:
#  1. Its codegen rejects the raw-ISA EVENT_SEMAPHORE_RANGE_CLEAR that the
#     bass kernel tail emits ("ISA wrong length") -> skip it. The DMA-state
#     drain (classic InstDrain with is_reset_sema) is still emitted, and
#     repeated NEFF executions were verified to stay correct.
#  2. Its codegen allows only ONE sync-wait per instruction ("Too many sync
#     wait commands") -> hoist extra waits onto preceding same-engine NOPs
#     (semantically identical: the engine blocks on each wait in order).
# ---------------------------------------------------------------------------


def _noop_sem_clear(self, sem):
    return None


bass.BassGpSimd.sem_clear = _noop_sem_clear


def _split_multi_waits(nc):
    n = 0
    for fn in nc.m.functions:
        for bb in fn.blocks:
            new_insts = []
            for ins in bb.instructions:
                si = ins.sync_info
                if si is not None and len(si.on_wait) > 1:
                    waits = list(si.on_wait)
                    for w in waits[:-1]:
                        n += 1
                        nop = mybir.InstNoOp(
                            name=f"wsplit-{n}", engine=ins.engine,
                            ins=[], outs=[])
                        nop.sync_info = bass_rust.SyncInfo(
                            on_wait=[w], on_update=[])
                        new_insts.append(nop)
                    ins.sync_info = bass_rust.SyncInfo(
                        on_wait=[waits[-1]], on_update=list(si.on_update))
                new_insts.append(ins)
            bb.instructions = new_insts
    return n


P = 128
N = 1024
D = 64
HID = 32
E = 128
H = 4
B = 8
NJ = N // P            # 8 j-chunks
DT = mybir.dt.float32
AF = mybir.ActivationFunctionType
AX = mybir.AxisListType
NEG = 0.2
EPS = 1e-5


# --------------------------------------------------------------------------
# Device program (one core, one batch element).
# zero_ij: list of (i, j) positions where adj==0 for any core (masked).
# --------------------------------------------------------------------------

def _emit(nc: bass.Bass, zero_ij):
    d_x = nc.dram_tensor("x", [N, D], DT, kind="ExternalInput")
    d_adj = nc.dram_tensor("adj", [N, N], DT, kind="ExternalInput")
    d_W1 = nc.dram_tensor("W1", [D, H * HID], DT, kind="ExternalInput")
    d_a1 = nc.dram_tensor("a1", [H, 2 * HID + 1], DT, kind="ExternalInput")
    d_r1 = nc.dram_tensor("res1_w", [D, H * HID], DT, kind="ExternalInput")
    d_ln1g = nc.dram_tensor("ln1_g", [1, H * HID], DT, kind="ExternalInput")
    d_ln1b = nc.dram_tensor("ln1_b", [1, H * HID], DT, kind="ExternalInput")
    d_W2 = nc.dram_tensor("W2", [H * HID, H * E], DT, kind="ExternalInput")
    d_a2 = nc.dram_tensor("a2", [H, 2 * E + 1], DT, kind="ExternalInput")
    d_r2 = nc.dram_tensor("res2_w", [H * HID, E], DT, kind="ExternalInput")
    d_ln2g = nc.dram_tensor("ln2_g", [1, E], DT, kind="ExternalInput")
    d_ln2b = nc.dram_tensor("ln2_b", [1, E], DT, kind="ExternalInput")
    d_W3 = nc.dram_tensor("W3", [E, H * E], DT, kind="ExternalInput")
    d_a3 = nc.dram_tensor("a3", [H, 2 * E + 1], DT, kind="ExternalInput")
    d_ln3g = nc.dram_tensor("ln3_g", [1, E], DT, kind="ExternalInput")
    d_ln3b = nc.dram_tensor("ln3_b", [1, E], DT, kind="ExternalInput")
    d_pw = nc.dram_tensor("pool_w", [E, 1], DT, kind="ExternalInput")
    d_pb = nc.dram_tensor("pool_b", [1, 1], DT, kind="ExternalInput")
    d_eye = nc.dram_tensor("ident128", [P, P], DT, kind="ExternalInput")

    d_Zt = nc.dram_tensor("Zt", [E, N], DT, kind="ExternalOutput")
    d_g = nc.dram_tensor("gvec", [E, 1], DT, kind="ExternalOutput")
    d_expe = nc.dram_tensor("expEt", [H * N, N], DT, kind="ExternalOutput")
    d_s = nc.dram_tensor("ssum", [H, N], DT, kind="ExternalOutput")

    with tile.TileContext(nc) as tc, ExitStack() as ctx:
        consts = ctx.enter_context(tc.tile_pool(name="consts", bufs=1))
        wpool = ctx.enter_context(tc.tile_pool(name="wpool", bufs=2))
        expp = ctx.enter_context(tc.tile_pool(name="expp", bufs=3))
        sijp = ctx.enter_context(tc.tile_pool(name="sijp", bufs=2))
        vpool = ctx.enter_context(tc.tile_pool(name="vpool", bufs=3))
        tmp = ctx.enter_context(tc.tile_pool(name="tmp", bufs=3))
        rep2 = ctx.enter_context(tc.tile_pool(name="rep2", bufs=4))
        rep1 = ctx.enter_context(tc.tile_pool(name="rep1", bufs=2))
        rows = ctx.enter_context(tc.tile_pool(name="rows", bufs=8))
        srows = ctx.enter_context(tc.tile_pool(name="srows", bufs=3))
        outw = ctx.enter_context(tc.tile_pool(name="outw", bufs=2))
        hpool = ctx.enter_context(tc.tile_pool(name="hpool", bufs=1))
        hTp = ctx.enter_context(tc.tile_pool(name="hTp", bufs=2))

        ident = consts.tile([P, P], DT)
        nc.sync.dma_start(ident, d_eye.ap())
        ones_col = consts.tile([P, 1], DT)
        nc.vector.memset(ones_col, 1.0)
        ones_row = consts.tile([1, P], DT)
        nc.vector.memset(ones_row, 1.0)
        eps_c = consts.tile([1, 1], DT)
        nc.vector.memset(eps_c, EPS)
        negones_row = consts.tile([1, N], DT)
        nc.vector.memset(negones_row, -1.0)

        def tr(out_ap, in_ap):
            p = in_ap.shape[0]
            nc.tensor.transpose(out_ap, in_ap, ident[0:p, 0:p])

        # ---- parameters ---------------------------------------------
        w1 = consts.tile([D, H * HID], DT)
        nc.sync.dma_start(w1, d_W1.ap())
        w2 = consts.tile([H * HID, H * E], DT)
        nc.sync.dma_start(w2, d_W2.ap())
        w3 = consts.tile([E, H * E], DT)
        nc.sync.dma_start(w3, d_W3.ap())
        r1 = consts.tile([D, H * HID], DT)
        nc.sync.dma_start(r1, d_r1.ap())
        r2 = consts.tile([H * HID, E], DT)
        nc.sync.dma_start(r2, d_r2.ap())
        pw = consts.tile([E, 1], DT)
        nc.sync.dma_start(pw, d_pw.ap())
        pb = consts.tile([1, 1], DT)
        nc.sync.dma_start(pb, d_pb.ap())
        a_rows = []
        for li, (d_a, fh) in enumerate(((d_a1, HID), (d_a2, E), (d_a3, E))):
            per_h = []
            for hh in range(H):
                t = consts.tile([1, 2 * fh + 1], DT, name=f"arow{li}_{hh}",
                                tag=f"arow{li}_{hh}")
                nc.sync.dma_start(t, d_a[hh:hh + 1, :])
                per_h.append(t)
            a_rows.append(per_h)
        ln_lhsT = []
        for li, (d_g_, d_b_) in enumerate(
                ((d_ln1g, d_ln1b), (d_ln2g, d_ln2b), (d_ln3g, d_ln3b))):
            t = consts.tile([2, P], DT, tag=f"ln{li}")
            nc.sync.dma_start(t[0:1, :], d_g_.ap())
            nc.sync.dma_start(t[1:2, :], d_b_.ap())
            ln_lhsT.append(t)

        # ---- prelude: transposes (adjT, xT, W^T, cols) --------------
        bt = consts.tile([P, NJ, N], DT)          # adjT: [j-in-chunk, jc, i]
        xT = hTp.tile([D, N], DT, name="xT", tag="hT")
        w1t = consts.tile([HID, H, D], DT)        # W1^T per-head chunks
        w2t = consts.tile([P, H, H * HID], DT)    # W2^T as 4 chunks [128,128]
        w3t = consts.tile([P, H, E], DT)
        g_cols = consts.tile([P, 3], DT)          # gamma columns per layer
        ae_cols = consts.tile([P, 3 * H], DT)     # a_e replicated per (L,h)
        acols = [consts.tile([fh, 2 * H], DT, name=f"acols{li}",
                             tag=f"ac{li}")
                 for li, fh in enumerate((HID, E, E))]

        with tc.tile_pool(name="prelsb", bufs=2) as prel, \
             tc.tile_pool(name="prelps", bufs=4, space="PSUM") as prelps:
            for ic in range(NJ):
                ach = prel.tile([P, N], DT, tag="ach", bufs=1)
                nc.sync.dma_start(ach, d_adj[ic * P:(ic + 1) * P, :])
                for jcb in range(NJ):
                    tp = prelps.tile([P, P], DT, tag="tp")
                    tr(tp, ach[:, jcb * P:(jcb + 1) * P])
                    nc.vector.tensor_copy(bt[:, jcb, ic * P:(ic + 1) * P], tp)
            for ncc in range(NJ):
                xch = prel.tile([P, D], DT, tag="xch")
                nc.sync.dma_start(xch, d_x[ncc * P:(ncc + 1) * P, :])
                tp = prelps.tile([D, P], DT, tag="tp")
                tr(tp, xch)
                nc.vector.tensor_copy(xT[:, ncc * P:(ncc + 1) * P], tp)
            for hh in range(H):
                tp = prelps.tile([HID, D], DT, tag="tp")
                tr(tp, w1[:, hh * HID:(hh + 1) * HID])
                nc.vector.tensor_copy(w1t[:, hh, :], tp)
            for w_, wt_ in ((w2, w2t), (w3, w3t)):
                for hh in range(H):
                    tp = prelps.tile([P, P], DT, tag="tp")
                    tr(tp, w_[:, hh * P:(hh + 1) * P])
                    nc.vector.tensor_copy(wt_[:, hh, :], tp)
            for li in range(3):
                tp = prelps.tile([P, 1], DT, tag="col")
                tr(tp, ln_lhsT[li][0:1, :])
                nc.vector.tensor_copy(g_cols[:, li:li + 1], tp)
                fh = (HID, E, E)[li]
                for hh in range(H):
                    for k, off in ((0, 0), (1, fh)):
                        tp = prelps.tile([fh, 1], DT, tag="col")
                        tr(tp, a_rows[li][hh][0:1, off:off + fh])
                        nc.vector.tensor_copy(
                            acols[li][0:fh, 2 * hh + k:2 * hh + k + 1], tp)
                    # a_e replicated to [128,1] via rank-1 outer product
                    tp2 = prelps.tile([P, 1], DT, tag="col")
                    nc.tensor.matmul(
                        tp2, ones_row,
                        a_rows[li][hh][0:1, 2 * fh:2 * fh + 1],
                        start=True, stop=True)
                    nc.vector.tensor_copy(
                        ae_cols[:, li * H + hh:li * H + hh + 1], tp2)

        # per-zero-position indicators iznz[k] = (adj[i,j] != 0), packed
        zpool = ctx.enter_context(tc.tile_pool(name="zpool", bufs=1))
        zpatch = ctx.enter_context(tc.tile_pool(name="zpatch", bufs=2))
        nz = max(1, len(zero_ij))
        zt_all = zpool.tile([1, nz], DT, name="zt_all", tag="zt_all")
        iznz_all = zpool.tile([1, nz], DT, name="iznz_all", tag="iznz_all")
        for k, (zi, zj) in enumerate(zero_ij):
            nc.sync.dma_start(zt_all[0:1, k:k + 1],
                              d_adj[zi:zi + 1, zj:zj + 1])
        nc.vector.tensor_scalar(iznz_all, zt_all, 0.0, None,
                                op0=alu.not_equal)

        # ------------------------------------------------------------------
        def gat_layer(li, prevT, f_in, fh, w_sb, wt_head, res_w, concat,
                      h_outT, emit_alpha, do_elu):
            """One GAT layer. prevT: [f_in, N] sbuf tile. Writes h_outT."""
            f_all = H * fh
            si_reps = []
            sj_cols = []
            hN = hpool.tile([P, NJ, f_all], DT, tag="hN")

            with tc.tile_pool(name=f"prp{li}", bufs=1, space="PSUM") as prp, \
                 tc.tile_pool(name=f"prs{li}", bufs=1, space="PSUM") as prs:
                # features hN = (prev @ W) natural layout, chunked over n
                for ncc in range(NJ):
                    fps = prp.tile([P, f_all], DT, tag="feat")
                    nc.tensor.matmul(
                        fps, prevT[:, ncc * P:(ncc + 1) * P], w_sb,
                        start=True, stop=True)
                    nc.scalar.copy(hN[:, ncc, :], fps)
                # attention score row-vectors s_i, s_j per head
                for hh in range(H):
                    srow_pair = []
                    for k in range(2):          # 0 = a_i, 1 = a_j
                        warow_ps = prs.tile([1, f_in], DT, tag="row")
                        nc.tensor.matmul(
                            warow_ps,
                            acols[li][0:fh, 2 * hh + k:2 * hh + k + 1],
                            wt_head(hh), start=True, stop=True)
                        warow = rows.tile([1, f_in], DT, tag="row", bufs=4)
                        nc.vector.tensor_copy(warow, warow_ps)
                        wacol_ps = prs.tile([f_in, 1], DT, tag="wacol")
                        tr(wacol_ps, warow)
                        wacol = rows.tile([f_in, 1], DT, tag="row", bufs=4)
                        nc.vector.tensor_copy(wacol, wacol_ps)
                        srow_ps = prs.tile([1, N], DT, tag="srow")
                        for hf in range(2):
                            nc.tensor.matmul(
                                srow_ps[:, hf * 512:(hf + 1) * 512], wacol,
                                prevT[:, hf * 512:(hf + 1) * 512],
                                start=True, stop=True)
                        srow = srows.tile([1, N], DT, tag="srow_sb")
                        nc.vector.tensor_copy(srow, srow_ps)
                        srow_pair.append(srow)
                    # si replicated to [128, N] via PE rank-1 outer product
                    sirep_ps = prp.tile([P, N], DT, tag="sirep")
                    for hf in range(2):
                        nc.tensor.matmul(
                            sirep_ps[:, hf * 512:(hf + 1) * 512], ones_row,
                            srow_pair[0][:, hf * 512:(hf + 1) * 512],
                            start=True, stop=True)
                    si_rep = rep2.tile([P, N], DT, name="si_rep", tag="sirep")
                    nc.scalar.copy(si_rep, sirep_ps)
                    si_reps.append(si_rep)
                    # s_j as [128,1] per-partition columns, per j-chunk
                    sjc_ps = prs.tile([P, NJ], DT, tag="sjc")
                    for jc in range(NJ):
                        tr(sjc_ps[:, jc:jc + 1],
                           srow_pair[1][0:1, jc * P:(jc + 1) * P])
                    sjc = srows.tile([P, NJ], DT, tag="sjc_sb", bufs=4)
                    nc.vector.tensor_copy(sjc, sjc_ps)
                    sj_cols.append(sjc)

            # ---- main attention loop --------------------------------
            with tc.tile_pool(name=f"agg{li}", bufs=2, space="PSUM") as aggp, \
                 tc.tile_pool(name=f"sps{li}", bufs=2, space="PSUM") as sps:
                # residual into outT (agg-tag slot, freed before heads run)
                outT = outw.tile([P, N], DT, tag="outT")
                if res_w is None:
                    nc.vector.tensor_copy(outT, prevT)
                else:
                    rps = aggp.tile([P, N], DT, tag="agg")
                    for hf in range(2):
                        nc.tensor.matmul(
                            rps[:, hf * 512:(hf + 1) * 512], res_w,
                            prevT[:, hf * 512:(hf + 1) * 512],
                            start=True, stop=True)
                    nc.scalar.copy(outT, rps)

                for hh in range(H):
                    si_rep = si_reps[hh]
                    s_ps = sps.tile([1, N], DT, name="s_ps", tag="s")
                    agg_ps = aggp.tile([fh, N], DT, name="agg_ps", tag="agg")
                    fsl = slice(hh * fh, (hh + 1) * fh)
                    osl = slice(hh * fh, (hh + 1) * fh) if concat \
                        else slice(0, fh)
                    aec = ae_cols[:, li * H + hh:li * H + hh + 1]
                    for jc in range(NJ):
                        sij = sijp.tile([P, N], DT, tag="sij")
                        nc.gpsimd.tensor_scalar(
                            sij, si_rep, sj_cols[hh][:, jc:jc + 1], None,
                            op0=alu.add)
                        w_t = wpool.tile([P, N], DT, name="w_t", tag="w", bufs=3)
                        nc.vector.scalar_tensor_tensor(
                            w_t, bt[:, jc, :], aec, sij,
                            op0=alu.mult, op1=alu.add)
                        nc.vector.scalar_tensor_tensor(
                            w_t, w_t, NEG, w_t, op0=alu.mult, op1=alu.max)
                        ex = expp.tile([P, N], DT, tag="ex")
                        nc.scalar.activation(ex, w_t, AF.Exp)
                        # exact adj==0 masking: ex *= (adj != 0), per core
                        for k, (zi, zj) in enumerate(zero_ij):
                            if zj // P == jc:
                                e_k = zpatch.tile([1, 1], DT, name="e_k",
                                                  tag="e_k")
                                nc.sync.dma_start(
                                    e_k, ex[zj % P:zj % P + 1, zi:zi + 1])
                                em_k = zpatch.tile([1, 1], DT, name="em_k",
                                                   tag="e_k")
                                nc.vector.tensor_tensor(
                                    em_k, e_k, iznz_all[0:1, k:k + 1],
                                    op=alu.mult)
                                nc.sync.dma_start(
                                    ex[zj % P:zj % P + 1, zi:zi + 1], em_k)
                        for hf in range(2):
                            sl = slice(hf * 512, (hf + 1) * 512)
                            nc.tensor.matmul(
                                s_ps[:, sl], ones_col, ex[:, sl],
                                start=(jc == 0), stop=(jc == NJ - 1))
                        for hf in range(2):
                            sl = slice(hf * 512, (hf + 1) * 512)
                            nc.tensor.matmul(
                                agg_ps[:, sl], hN[:, jc, fsl], ex[:, sl],
                                start=(jc == 0), stop=(jc == NJ - 1))
                        if emit_alpha:
                            nc.sync.dma_start(
                                d_expe[hh * N + jc * P:hh * N + (jc + 1) * P,
                                       :], ex)
                    # 1/s (and /H for the head-mean) via exp(-ln(scale*s))
                    lns = rows.tile([1, N], DT, tag="row", bufs=4)
                    nc.scalar.activation(
                        lns, s_ps, AF.Ln, scale=(1.0 if concat else float(H)))
                    srec = rows.tile([1, N], DT, tag="row", bufs=4)
                    nc.scalar.activation(srec, lns, AF.Exp, scale=-1.0)
                    if emit_alpha:
                        s_sb = rows.tile([1, N], DT, tag="row", bufs=4)
                        nc.vector.tensor_copy(s_sb, s_ps)
                        nc.sync.dma_start(d_s[hh:hh + 1, :], s_sb)
                    srecrep_ps = sps.tile([P, N], DT, name="srecrep_ps",
                                          tag="s")
                    for hf in range(2):
                        nc.tensor.matmul(
                            srecrep_ps[:, hf * 512:(hf + 1) * 512], ones_row,
                            srec[:, hf * 512:(hf + 1) * 512],
                            start=True, stop=True)
                    sr_rep = rep1.tile([P, N], DT, name="sr_rep", tag="rep1b")
                    nc.scalar.copy(sr_rep, srecrep_ps)
                    v_t = vpool.tile([fh, N], DT, name="v_t", tag="v")
                    nc.vector.tensor_tensor(
                        v_t, agg_ps, sr_rep[0:fh, :], op=alu.mult)
                    if concat:
                        # concat head output into outT rows via SBUF DMA
                        # (cross-partition-offset compute ops are illegal)
                        res32 = vpool.tile([fh, N], DT, name="res32", tag="v")
                        nc.sync.dma_start(res32, outT[osl, :])
                        o32 = vpool.tile([fh, N], DT, name="o32", tag="v")
                        nc.vector.tensor_tensor(o32, v_t, res32, op=alu.add)
                        nc.sync.dma_start(outT[osl, :], o32)
                    else:
                        nc.gpsimd.tensor_tensor(
                            outT, outT, v_t, op=alu.add)

            # ---- layernorm over features (partition dim) ------------
            with tc.tile_pool(name=f"lnp{li}", bufs=1, space="PSUM") as lnp:
                sq = tmp.tile([P, N], DT, tag="tmp")
                nc.gpsimd.tensor_tensor(sq, outT, outT, op=alu.mult)
                st1_ps = lnp.tile([1, N], DT, tag="st1")
                st2_ps = lnp.tile([1, N], DT, tag="st2")
                for hf in range(2):
                    sl = slice(hf * 512, (hf + 1) * 512)
                    nc.tensor.matmul(st1_ps[:, sl], ones_col, outT[:, sl],
                                     start=True, stop=True)
                    nc.tensor.matmul(st2_ps[:, sl], ones_col, sq[:, sl],
                                     start=True, stop=True)
                mu = rows.tile([1, N], DT, tag="row", bufs=4)
                nc.vector.tensor_scalar(mu, st1_ps, 1.0 / P, None,
                                        op0=alu.mult)
                musq = rows.tile([1, N], DT, tag="row", bufs=4)
                nc.vector.tensor_tensor(musq, mu, mu, op=alu.mult)
                var = rows.tile([1, N], DT, tag="row", bufs=4)
                nc.vector.scalar_tensor_tensor(
                    var, st2_ps, 1.0 / P, musq,
                    op0=alu.mult, op1=alu.subtract)
                varc = rows.tile([1, N], DT, tag="row", bufs=4)
                nc.vector.tensor_scalar_max(varc, var, 0.0)
                sd = rows.tile([1, N], DT, tag="row", bufs=4)
                nc.scalar.activation(sd, varc, AF.Sqrt, bias=eps_c)
                lnsd = rows.tile([1, N], DT, tag="row", bufs=4)
                nc.scalar.activation(lnsd, sd, AF.Ln)
                rstd = rows.tile([1, N], DT, tag="row", bufs=4)
                nc.scalar.activation(rstd, lnsd, AF.Exp, scale=-1.0)
                r2rhs = rows.tile([2, N], DT, tag="r2rhs", bufs=1)
                nc.vector.tensor_tensor(r2rhs[0:1, :], mu, rstd, op=alu.mult)
                nc.sync.dma_start(r2rhs[1:2, :], negones_row)
                R_ps = lnp.tile([P, N], DT, tag="R")
                for hf in range(2):
                    sl = slice(hf * 512, (hf + 1) * 512)
                    nc.tensor.matmul(R_ps[:, sl], ln_lhsT[li], r2rhs[:, sl],
                                     start=True, stop=True)
                rstdrep_ps = lnp.tile([P, N], DT, tag="rrep")
                for hf in range(2):
                    nc.tensor.matmul(
                        rstdrep_ps[:, hf * 512:(hf + 1) * 512], ones_row,
                        rstd[:, hf * 512:(hf + 1) * 512],
                        start=True, stop=True)
                rstd_rep = rep1.tile([P, N], DT, name="rstd_rep", tag="rep1b")
                nc.scalar.copy(rstd_rep, rstdrep_ps)
                v1 = tmp.tile([P, N], DT, tag="tmp")
                nc.vector.tensor_tensor(v1, outT, rstd_rep, op=alu.mult)
                ln_out = h_outT if not do_elu else tmp.tile(
                    [P, N], DT, name="ln_out", tag="tmp")
                nc.vector.scalar_tensor_tensor(
                    ln_out, v1, g_cols[:, li:li + 1], R_ps,
                    op0=alu.mult, op1=alu.subtract)
                if do_elu:
                    rl = tmp.tile([P, N], DT, tag="tmp")
                    nc.vector.tensor_scalar_max(rl, ln_out, 0.0)
                    mn = tmp.tile([P, N], DT, tag="tmp")
                    nc.vector.tensor_scalar_min(mn, ln_out, 0.0)
                    ee = tmp.tile([P, N], DT, name="ee", tag="tmp")
                    nc.scalar.activation(ee, mn, AF.Exp)
                    nc.vector.scalar_tensor_tensor(
                        h_outT, ee, -1.0, rl, op0=alu.add, op1=alu.add)

        # ---- run the three layers -----------------------------------
        h1T = hTp.tile([P, N], DT, name="h1T", tag="hT")
        h2T = hTp.tile([P, N], DT, name="h2T", tag="hT")
        zT = hTp.tile([P, N], DT, name="zT", tag="hT")
        gat_layer(0, xT, D, HID, w1,
                  lambda hh: w1t[:, hh, :], r1, True, h1T, False, True)
        gat_layer(1, h1T, H * HID, E, w2,
                  lambda hh: w2t[:, hh, :], r2, False, h2T, False, False)
        gat_layer(2, h2T, E, E, w3,
                  lambda hh: w3t[:, hh, :], None, False, zT, True, False)

        # ---- attention pooling --------------------------------------
        with tc.tile_pool(name="poolp", bufs=1, space="PSUM") as pp:
            sc_ps = pp.tile([1, N], DT, tag="sc")
            for hf in range(2):
                sl = slice(hf * 512, (hf + 1) * 512)
                nc.tensor.matmul(sc_ps[:, sl], pw, zT[:, sl],
                                 start=True, stop=True)
            sc = rows.tile([1, N], DT, tag="row", bufs=4)
            nc.vector.tensor_scalar(sc, sc_ps, pb[0:1, 0:1], None, op0=alu.add)
            mx = rows.tile([1, 1], DT, tag="row", bufs=4)
            nc.vector.reduce_max(mx, sc, axis=AX.X)
            nmx = rows.tile([1, 1], DT, tag="row", bufs=4)
            nc.vector.tensor_scalar_mul(nmx, mx, -1.0)
            wrow = rows.tile([1, N], DT, tag="row", bufs=4)
            wsum = rows.tile([1, 1], DT, tag="row", bufs=4)
            nc.scalar.activation(wrow, sc, AF.Exp, bias=nmx, accum_out=wsum)
            wis = rows.tile([1, 1], DT, tag="row", bufs=4)
            nc.vector.reciprocal(wis, wsum)
            wnorm = rows.tile([1, N], DT, tag="row", bufs=4)
            nc.vector.tensor_scalar_mul(wnorm, wrow, wis)
            wrep_ps = pp.tile([P, N], DT, tag="wrep")
            for hf in range(2):
                nc.tensor.matmul(
                    wrep_ps[:, hf * 512:(hf + 1) * 512], ones_row,
                    wnorm[:, hf * 512:(hf + 1) * 512], start=True, stop=True)
            w_rep = rep1.tile([P, N], DT, name="w_rep", tag="rep1b")
            nc.scalar.copy(w_rep, wrep_ps)
            gsc = tmp.tile([P, N], DT, name="gsc", tag="tmp")
            nc.vector.tensor_tensor(gsc, zT, w_rep, op=alu.mult)
            gcol = rows.tile([P, 1], DT, tag="row", bufs=4)
            nc.vector.reduce_sum(gcol, gsc, axis=AX.X)
            nc.sync.dma_start(d_g.ap(), gcol)
            nc.sync.dma_start(d_Zt.ap(), zT)

    return nc


# --------------------------------------------------------------------------
# Host entry point
# --------------------------------------------------------------------------

_CACHE = {}


def _get_nc(zero_ij, split=True):
    key = (tuple(sorted(zero_ij)), split)
    if key not in _CACHE:
        nc = bass.Bass("TRN2", target_bir_lowering=False, debug=False)
        _emit(nc, sorted(zero_ij))
        if split:
            _split_multi_waits(nc)
        _CACHE[key] = nc
    return _CACHE[key]


def _in_map(inputs, b):
    f32 = lambda a: np.ascontiguousarray(np.asarray(a, dtype=np.float32))
    return {
        "x": f32(inputs["x"][b]),
        "adj": f32(inputs["adj"][b]),
        "W1": f32(inputs["W1"]),
        "a1": f32(inputs["a1"]),
        "res1_w": f32(inputs["res1_w"]),
        "ln1_g": f32(inputs["ln1_g"]).reshape(1, -1),
        "ln1_b": f32(inputs["ln1_b"]).reshape(1, -1),
        "W2": f32(inputs["W2"]),
        "a2": f32(inputs["a2"]),
        "res2_w": f32(inputs["res2_w"]),
        "ln2_g": f32(inputs["ln2_g"]).reshape(1, -1),
        "ln2_b": f32(inputs["ln2_b"]).reshape(1, -1),
        "W3": f32(inputs["W3"]),
        "a3": f32(inputs["a3"]),
        "ln3_g": f32(inputs["ln3_g"]).reshape(1, -1),
        "ln3_b": f32(inputs["ln3_b"]).reshape(1, -1),
        "pool_w": f32(inputs["pool_w"]).reshape(E, 1),
        "pool_b": f32(inputs["pool_b"]).reshape(1, 1),
        "ident128": np.eye(P, dtype=np.float32),
    }


def _assemble(results):
    Z = np.empty((B, N, E), dtype=np.float32)
    g = np.empty((B, E), dtype=np.float32)
    alpha = np.empty((B, N, N, H), dtype=np.float32)
    for b in range(B):
        r = results[b]
        Z[b] = r["Zt"].T
        g[b] = r["gvec"][:, 0]
        ex = r["expEt"].reshape(H, N, N)   # [H, j, i]
        s = r["ssum"]                      # [H, i]
        denom = s.T[:, None, :]            # [i, 1, H]
        num = ex.transpose(2, 1, 0)        # [i, j, H]
        with np.errstate(divide="ignore", invalid="ignore"):
            al = num / denom
        alpha[b] = np.where(denom == 0.0, 0.0, al)
    return Z, g, alpha


def kernel(**inputs):
    from concourse.bass_utils import run_bass_kernel_spmd
    adj = np.asarray(inputs["adj"])
    # all cores run one SPMD program; bake the union of adj==0 positions
    zs = np.argwhere(adj == 0.0)
    zero_ij = sorted({(int(i), int(j)) for (_b, i, j) in zs})
    nc = _get_nc(zero_ij)
    in_maps = [_in_map(inputs, b) for b in range(B)]
    res = run_bass_kernel_spmd(nc, in_maps, list(range(B)))
    return _assemble(res.results)
